# revision 2
# baseline (speedup 1.0000x reference)
"""Trainium2 Bass kernel for 2-layer GraphSAGE node classification.

Strategy (data-parallel over the 4096 output seeds, no collectives):
- Replicate node_feat on all 8 cores.
- Each core computes 512 seeds' logits end-to-end. Working backwards, a core
  needs 5632 layer-0 rows (512 roots + 512*10 col1 neighbors, no dedup), each
  requiring 25 neighbor gathers + 1 root gather from node_feat => 146432
  row-gathers of 512B per core (8 * 146432 = N0 = exactly the reference's
  total gather volume).
- Gathers: SWDGE indirect DMA, 128 rows per instruction (HW limit: one index
  per output partition).
- Neighbor mean + transpose fused into PE matmuls: for a block of 128 layer-0
  rows, gather tiles G_j [128 gathered rows, 128 feats]; then
  nmT[d,u] = sum_j G_j^T @ S_j with constant selection matrices S_j
  (S_j[p,u] = 1/25 iff (j*128+p)//25 == u). Root transpose = G_25^T @ I.
- Layer 0: H[u,:] = relu(nmT^T @ Wn0 + rootT^T @ Wr0 + b0) via PE.
- Layer 1 slots are laid out so the 10-neighbor mean is another fixed
  S-matrix matmul (5 matrices, lcm(128,10)=640-periodic), roots via PE
  transpose; final [128,47] matmuls + bias.
"""

import numpy as np

N_NODES = 1_000_000
IN_DIM = 128
HID = 256
NCLS = 47
N0, N1, N2 = 1_171_456, 45_056, 4_096
F0, F1 = 25, 10

NCORES = 8
SEEDS_PER_CORE = N2 // NCORES  # 512
NU = SEEDS_PER_CORE * (1 + F1)  # 5632 layer-0 rows per core
P = 128
NB = NU // P  # 44 blocks
J = F0 + 1  # 26 gather columns per block (25 neighbors + root)
NSEED_TILES = SEEDS_PER_CORE // P  # 4
NGRP = SEEDS_PER_CORE // 64  # 8 groups of 64 seeds (640 slots = 5 tiles)

_CACHED = {}


def _build_program():
    if "nc" in _CACHED:
        return _CACHED["nc"]
    import concourse.bacc as bacc
    import concourse.bass as bass
    import concourse.mybir as mybir
    import concourse.tile as tile

    f32 = mybir.dt.float32
    i32 = mybir.dt.int32

    nc = bacc.Bacc("TRN2", target_bir_lowering=False, debug=False)

    node_feat = nc.dram_tensor("node_feat", [N_NODES, IN_DIM], f32, kind="ExternalInput")
    idx = nc.dram_tensor("idx", [NB, P, J], i32, kind="ExternalInput")
    smat = nc.dram_tensor("smat", [P, J * P], f32, kind="ExternalInput")
    s1mat = nc.dram_tensor("s1mat", [P, 5 * 64], f32, kind="ExternalInput")
    wn0 = nc.dram_tensor("wn0", [IN_DIM, HID], f32, kind="ExternalInput")
    wr0 = nc.dram_tensor("wr0", [IN_DIM, HID], f32, kind="ExternalInput")
    wn1 = nc.dram_tensor("wn1", [HID, NCLS], f32, kind="ExternalInput")
    wr1 = nc.dram_tensor("wr1", [HID, NCLS], f32, kind="ExternalInput")
    b0b = nc.dram_tensor("b0b", [P, HID], f32, kind="ExternalInput")
    b1b = nc.dram_tensor("b1b", [P, NCLS], f32, kind="ExternalInput")
    out = nc.dram_tensor("out", [SEEDS_PER_CORE, NCLS], f32, kind="ExternalOutput")

    with tile.TileContext(nc) as tc:
        with (
            tc.tile_pool(name="const", bufs=1) as cp,
            tc.tile_pool(name="idxp", bufs=4) as idxp,
            tc.tile_pool(name="gp", bufs=3) as gp,
            tc.tile_pool(name="sb", bufs=4) as sb,
            tc.tile_pool(name="psA", bufs=2, space="PSUM") as psA,
            tc.tile_pool(name="psB", bufs=2, space="PSUM") as psB,
        ):
            smat_t = cp.tile([P, J * P], f32)
            nc.sync.dma_start(out=smat_t[:], in_=smat[:])
            s1_t = cp.tile([P, 5 * 64], f32)
            nc.sync.dma_start(out=s1_t[:], in_=s1mat[:])
            wn0_t = cp.tile([P, HID], f32)
            nc.sync.dma_start(out=wn0_t[:], in_=wn0[:])
            wr0_t = cp.tile([P, HID], f32)
            nc.sync.dma_start(out=wr0_t[:], in_=wr0[:])
            wn1_t = cp.tile([P, 2 * NCLS], f32)
            nc.sync.dma_start(out=wn1_t[:, 0:NCLS], in_=wn1[0:P, :])
            nc.sync.dma_start(out=wn1_t[:, NCLS : 2 * NCLS], in_=wn1[P : 2 * P, :])
            wr1_t = cp.tile([P, 2 * NCLS], f32)
            nc.sync.dma_start(out=wr1_t[:, 0:NCLS], in_=wr1[0:P, :])
            nc.sync.dma_start(out=wr1_t[:, NCLS : 2 * NCLS], in_=wr1[P : 2 * P, :])
            b0_t = cp.tile([P, HID], f32)
            nc.sync.dma_start(out=b0_t[:], in_=b0b[:])
            b1_t = cp.tile([P, NCLS], f32)
            nc.sync.dma_start(out=b1_t[:], in_=b1b[:])

            # all 44 H tiles live in SBUF: block b at [:, b*HID:(b+1)*HID]
            H = cp.tile([P, NB * HID], f32)

            ident = smat_t[:, F0 * P : J * P]  # S_25 == identity

            for b in range(NB):
                idx_t = idxp.tile([P, J], i32, tag="idx")
                nc.sync.dma_start(out=idx_t[:], in_=idx[b])
                gt = gp.tile([P, J * P], f32, tag="g")
                for j in range(J):
                    nc.gpsimd.indirect_dma_start(
                        out=gt[:, j * P : (j + 1) * P],
                        out_offset=None,
                        in_=node_feat[:],
                        in_offset=bass.IndirectOffsetOnAxis(
                            ap=idx_t[:, j : j + 1], axis=0
                        ),
                    )
                nmT_ps = psA.tile([P, P], f32, tag="nmT")
                for j in range(F0):
                    nc.tensor.matmul(
                        out=nmT_ps[:],
                        lhsT=gt[:, j * P : (j + 1) * P],
                        rhs=smat_t[:, j * P : (j + 1) * P],
                        start=(j == 0),
                        stop=(j == F0 - 1),
                    )
                rT_ps = psA.tile([P, P], f32, tag="rT")
                nc.tensor.matmul(
                    out=rT_ps[:],
                    lhsT=gt[:, F0 * P : J * P],
                    rhs=ident,
                    start=True,
                    stop=True,
                )
                nmT = sb.tile([P, P], f32, tag="nmT_s")
                nc.vector.tensor_copy(out=nmT[:], in_=nmT_ps[:])
                rT = sb.tile([P, P], f32, tag="rT_s")
                nc.scalar.copy(out=rT[:], in_=rT_ps[:])
                h_ps = psB.tile([P, HID], f32, tag="h")
                nc.tensor.matmul(
                    out=h_ps[:], lhsT=nmT[:], rhs=wn0_t[:], start=True, stop=False
                )
                nc.tensor.matmul(
                    out=h_ps[:], lhsT=rT[:], rhs=wr0_t[:], start=False, stop=True
                )
                hs = H[:, b * HID : (b + 1) * HID]
                nc.vector.tensor_tensor(
                    out=hs, in0=h_ps[:], in1=b0_t[:], op=mybir.AluOpType.add
                )
                nc.scalar.activation(
                    out=hs, in_=hs, func=mybir.ActivationFunctionType.Relu
                )

            # ---- layer 1 ----
            x1mT = cp.tile([P, 2 * SEEDS_PER_CORE], f32)  # [h_half, seed]
            rtT = cp.tile([P, 2 * SEEDS_PER_CORE], f32)
            for g in range(NGRP):
                for half in range(2):
                    ps = psA.tile([P, 64], f32, tag="nmT")
                    for q in range(5):
                        tb = NSEED_TILES + g * 5 + q
                        nc.tensor.matmul(
                            out=ps[:],
                            lhsT=H[:, tb * HID + half * P : tb * HID + half * P + P],
                            rhs=s1_t[:, q * 64 : (q + 1) * 64],
                            start=(q == 0),
                            stop=(q == 4),
                        )
                    nc.vector.tensor_copy(
                        out=x1mT[
                            :,
                            half * SEEDS_PER_CORE + g * 64 : half * SEEDS_PER_CORE + (g + 1) * 64,
                        ],
                        in_=ps[:],
                    )
            for t in range(NSEED_TILES):
                for half in range(2):
                    ps = psA.tile([P, P], f32, tag="rT")
                    nc.tensor.matmul(
                        out=ps[:],
                        lhsT=H[:, t * HID + half * P : t * HID + half * P + P],
                        rhs=ident,
                        start=True,
                        stop=True,
                    )
                    nc.vector.tensor_copy(
                        out=rtT[
                            :,
                            half * SEEDS_PER_CORE + t * P : half * SEEDS_PER_CORE + (t + 1) * P,
                        ],
                        in_=ps[:],
                    )
            for t in range(NSEED_TILES):
                ps = psB.tile([P, NCLS], f32, tag="h")
                for half in range(2):
                    nc.tensor.matmul(
                        out=ps[:],
                        lhsT=x1mT[:, half * SEEDS_PER_CORE + t * P : half * SEEDS_PER_CORE + t * P + P],
                        rhs=wn1_t[:, half * NCLS : (half + 1) * NCLS],
                        start=(half == 0),
                        stop=False,
                    )
                for half in range(2):
                    nc.tensor.matmul(
                        out=ps[:],
                        lhsT=rtT[:, half * SEEDS_PER_CORE + t * P : half * SEEDS_PER_CORE + t * P + P],
                        rhs=wr1_t[:, half * NCLS : (half + 1) * NCLS],
                        start=False,
                        stop=(half == 1),
                    )
                ot = sb.tile([P, NCLS], f32, tag="ot")
                nc.vector.tensor_tensor(
                    out=ot[:], in0=ps[:], in1=b1_t[:], op=mybir.AluOpType.add
                )
                nc.sync.dma_start(out=out[t * P : (t + 1) * P, :], in_=ot[:])

    nc.compile()
    _CACHED["nc"] = nc
    return nc


def _host_prep(node_feat, gid0, col0, col1, Wn0, Wr0, b0, Wn1, Wr1, b1):
    """Build per-core input maps."""
    node_feat = np.ascontiguousarray(np.asarray(node_feat, dtype=np.float32))
    gid0 = np.asarray(gid0).astype(np.int64)
    col0 = np.asarray(col0).astype(np.int64).reshape(N1, F0)
    col1 = np.asarray(col1).astype(np.int64).reshape(N2, F1)

    # constant selection matrices
    smat = np.zeros((P, J * P), dtype=np.float32)
    for j in range(F0):
        e = j * P + np.arange(P)  # entry within 3200-block
        smat[np.arange(P), j * P + e // F0] = 1.0 / F0
    smat[np.arange(P), F0 * P + np.arange(P)] = 1.0  # identity for transposes

    s1 = np.zeros((P, 5 * 64), dtype=np.float32)
    for q in range(5):
        e = q * P + np.arange(P)  # slot within 640-group
        s1[np.arange(P), q * 64 + e // F1] = 1.0 / F1

    common = {
        "smat": smat,
        "s1mat": s1,
        "wn0": np.asarray(Wn0, dtype=np.float32),
        "wr0": np.asarray(Wr0, dtype=np.float32),
        "wn1": np.asarray(Wn1, dtype=np.float32),
        "wr1": np.asarray(Wr1, dtype=np.float32),
        "b0b": np.broadcast_to(np.asarray(b0, dtype=np.float32), (P, HID)).copy(),
        "b1b": np.broadcast_to(np.asarray(b1, dtype=np.float32), (P, NCLS)).copy(),
    }

    in_maps = []
    for c in range(NCORES):
        seeds = np.arange(c * SEEDS_PER_CORE, (c + 1) * SEEDS_PER_CORE)
        slot_rows = np.concatenate([seeds, col1[seeds].ravel()])  # [NU]
        nid = gid0[col0[slot_rows]]  # [NU, F0]
        rid = gid0[slot_rows]  # [NU]
        idx = np.empty((NB, P, J), dtype=np.int32)
        for b in range(NB):
            flat = nid[b * P : (b + 1) * P].ravel()  # [3200]
            idx[b, :, :F0] = flat.reshape(F0, P).T
            idx[b, :, F0] = rid[b * P : (b + 1) * P]
        in_maps.append({"node_feat": node_feat, "idx": idx, **common})
    return in_maps


def _run(in_maps, trace=False):
    from concourse.bass_utils import run_bass_kernel_spmd

    nc = _build_program()
    res = run_bass_kernel_spmd(
        nc, in_maps, core_ids=list(range(NCORES)), trace=trace
    )
    out = np.concatenate([r["out"] for r in res.results], axis=0)
    return out, res


def kernel(node_feat, gid0, col0, col1, Wn0, Wr0, b0, Wn1, Wr1, b1):
    in_maps = _host_prep(node_feat, gid0, col0, col1, Wn0, Wr0, b0, Wn1, Wr1, b1)
    out, _ = _run(in_maps, trace=False)
    return out


def run_traced(inputs):
    """test.py helper: returns (output, BassKernelResults with profile)."""
    in_maps = _host_prep(**inputs)
    return _run(in_maps, trace=True)
